# revision 1
# baseline (speedup 1.0000x reference)
"""Trainium2 Bass kernel for CausalSelfAttention (B=4, T=2048, C=768, H=6, D=128)
with RoPE + QK-RMSNorm.

Sharding: 8 cores = batch(4) x head-group(2, 3 heads each). Each core:
  - computes Q^T,K^T in (D, T) layout and V in (T, D) layout for its 3 heads
  - RoPE + RMSNorm on Q/K (partition-dim reductions via ones-matmul on PE)
  - causal attention with scores computed transposed (S^T: T_k on partitions,
    T_q on free dim) so softmax denom + AV matmuls need no transposes
  - partial c_proj over its 384 input channels
Host sums the two head-group partials per batch.
"""

import numpy as np

_B, _T, _C, _H, _D = 4, 2048, 768, 6, 128
_HPG = 3            # heads per group
_HD = _HPG * _D     # 384, per-group head dims
_NT = 4             # T tiles of 512
_TW = 512           # tile width (T_q)
_NKC = _T // 128    # 16 k-chunks of 128
_NCB = _C // 128    # 6 c_in chunks
_EPS = 1e-15

_cached = {}


def _build_nc():
    from contextlib import ExitStack
    from concourse import bacc, tile, mybir

    f32 = mybir.dt.float32
    f32r = mybir.dt.float32r
    Act = mybir.ActivationFunctionType
    Op = mybir.AluOpType

    nc = bacc.Bacc("TRN2", target_bir_lowering=False, debug=False)

    xT = nc.dram_tensor("xT", (_C, _T), f32r, kind="ExternalInput").ap()
    wq = nc.dram_tensor("wq", (_C, _HD), f32r, kind="ExternalInput").ap()
    wk = nc.dram_tensor("wk", (_C, _HD), f32r, kind="ExternalInput").ap()
    wv = nc.dram_tensor("wv", (_C, _HD), f32r, kind="ExternalInput").ap()
    wo = nc.dram_tensor("wo", (_HD, _C), f32r, kind="ExternalInput").ap()
    cc = nc.dram_tensor("cc", (128, _T), f32r, kind="ExternalInput").ap()
    ss = nc.dram_tensor("ss", (128, _T), f32r, kind="ExternalInput").ap()
    tri = nc.dram_tensor("tri", (128, 128), f32r, kind="ExternalInput").ap()
    ones = nc.dram_tensor("ones", (128, 128), f32r, kind="ExternalInput").ap()
    perm = nc.dram_tensor("perm", (128, 128), f32r, kind="ExternalInput").ap()
    out = nc.dram_tensor("out", (_T, _C), f32, kind="ExternalOutput").ap()

    with tile.TileContext(nc) as tc, ExitStack() as ctx, \
            nc.allow_low_precision(reason="f32r tiles carry full fp32 bits; PE rounds at ingest"):
        # --- pools ---
        pc = ctx.enter_context(tc.tile_pool(name="pc", bufs=1))
        pg = ctx.enter_context(tc.tile_pool(name="pg", bufs=2))         # Q tile scratch
        pa = ctx.enter_context(tc.tile_pool(name="pa", bufs=4))         # A chunks
        pz = ctx.enter_context(tc.tile_pool(name="pz", bufs=6))         # Z chunks
        psm = ctx.enter_context(tc.tile_pool(name="psm", bufs=2))       # small (1,512)/(128,512)
        pob = ctx.enter_context(tc.tile_pool(name="pob", bufs=2))       # out staging
        # psum pools (8 banks total)
        ppq = ctx.enter_context(tc.tile_pool(name="ppq", bufs=2, space="PSUM"))
        pps = ctx.enter_context(tc.tile_pool(name="pps", bufs=2, space="PSUM"))
        ppo = ctx.enter_context(tc.tile_pool(name="ppo", bufs=2, space="PSUM"))
        ppd = ctx.enter_context(tc.tile_pool(name="ppd", bufs=1, space="PSUM"))
        ppm = ctx.enter_context(tc.tile_pool(name="ppm", bufs=1, space="PSUM"))

        # --- constants / inputs resident in SBUF ---
        # load order matters: wk + xT feed the first PE work (K-projections);
        # cc/ss are not needed until rope, wq not until phase B, wo until c_proj
        t_wq, t_wk, t_wv = [], [], []
        for c in range(_NCB):
            t = pc.tile([128, _HD], f32r, tag=f"wk{c}", name=f"wk{c}",
                        padded_shape=[128, _TW])
            nc.sync.dma_start(t[:], wk[c * 128:(c + 1) * 128, :])
            t_wk.append(t)
        t_xt = []
        for c in range(_NCB):
            t = pc.tile([128, _T], f32r, tag=f"xt{c}", name=f"xt{c}")
            nc.sync.dma_start(t[:], xT[c * 128:(c + 1) * 128, :])
            t_xt.append(t)
        for c in range(_NCB):
            t = pc.tile([128, _HD], f32r, tag=f"wv{c}", name=f"wv{c}",
                        padded_shape=[128, _TW])
            nc.sync.dma_start(t[:], wv[c * 128:(c + 1) * 128, :])
            t_wv.append(t)
        t_cc = pc.tile([128, _T], f32r, tag="cc")
        t_ss = pc.tile([128, _T], f32r, tag="ss")
        nc.sync.dma_start(t_cc[:], cc[:])
        nc.sync.dma_start(t_ss[:], ss[:])
        for c in range(_NCB):
            t = pc.tile([128, _HD], f32r, tag=f"wq{c}", name=f"wq{c}")
            nc.sync.dma_start(t[:], wq[c * 128:(c + 1) * 128, :])
            t_wq.append(t)
        t_tri = pc.tile([128, 128], f32r, tag="tri")
        t_ones = pc.tile([128, 128], f32r, tag="ones")
        t_perm = pc.tile([128, 128], f32r, tag="perm")
        nc.sync.dma_start(t_tri[:], tri[:])
        nc.sync.dma_start(t_ones[:], ones[:])
        nc.sync.dma_start(t_perm[:], perm[:])
        t_ones_col = t_ones[:, 0:1]
        t_ones_row = t_ones[0:1, :]
        t_eps = pc.tile([128, 1], f32, tag="eps")
        nc.gpsimd.memset(t_eps[:], _EPS)
        t_wo = []
        for c in range(_HPG):
            t = pc.tile([128, _C], f32r, tag=f"wo{c}", name=f"wo{c}")
            nc.sync.dma_start(t[:], wo[c * 128:(c + 1) * 128, :])
            t_wo.append(t)

        # persistent K^T (post rope+norm) per head, and V blocks
        t_kn = [pc.tile([128, _T], f32r, tag=f"kn{h}", name=f"kn{h}") for h in range(_HPG)]
        t_v = [pc.tile([128, _HD], f32r, tag=f"v{tb}", name=f"v{tb}") for tb in range(_NKC)]

        def rope_part(dst_ap, col0):
            """In-place RoPE on dst_ap (128, 512)."""
            csl = slice(col0, col0 + _TW)
            p_sw = pps.tile([128, _TW], f32, tag="ps", name="p_sw")
            nc.tensor.matmul(p_sw[:], t_perm[:], dst_ap, start=True, stop=True)
            t_sw = pg.tile([128, _TW], f32r, tag="sw512", name="sw512", bufs=3)
            nc.vector.tensor_mul(dst_ap, dst_ap, t_cc[:, csl])
            nc.vector.tensor_mul(t_sw[:], p_sw[:], t_ss[:, csl])
            nc.vector.tensor_add(dst_ap, dst_ap, t_sw[:])

        def norm_pre(dst_ap, bc_pool, bc_tag, ms_on_act=True):
            """Square + partition-sum + broadcast; returns bcast psum."""
            t_sq = pg.tile([128, _TW], f32r, tag="sq512", name="sq512", bufs=3)
            nc.vector.tensor_mul(t_sq[:], dst_ap, dst_ap)
            p_ms = ppm.tile([1, _TW], f32, tag="pms", name="p_ms")
            nc.tensor.matmul(p_ms[:], t_ones_col, t_sq[:], start=True, stop=True)
            t_ms = psm.tile([1, _TW], f32r, tag="ms", name="t_ms", bufs=3)
            if ms_on_act:
                nc.scalar.copy(t_ms[:], p_ms[:])
            else:
                nc.vector.tensor_copy(t_ms[:], p_ms[:])
            p_bc = bc_pool.tile([128, _TW], f32, tag=bc_tag, name="p_bc")
            nc.tensor.matmul(p_bc[:], t_ones_row, t_ms[:], start=True, stop=True)
            return p_bc

        def norm_post(dst_ap, p_bc):
            """sqrt -> reciprocal -> scale, in place on dst_ap."""
            t_sd = psm.tile([128, _TW], f32r, tag="sd", name="t_sd", bufs=3)
            nc.scalar.activation(t_sd[:], p_bc[:], Act.Sqrt,
                                 bias=t_eps[:], scale=1.0 / 128.0)
            nc.vector.reciprocal(t_sd[:], t_sd[:])
            nc.vector.tensor_mul(dst_ap, dst_ap, t_sd[:])

        # one bcast-psum route per head so three chains can be in flight
        _bc_routes = [(pps, "ps"), (ppq, "pq"), (ppd, "pd")]

        def rope_norm(dst_ap, tw, col0):
            rope_part(dst_ap, col0)
            norm_post(dst_ap, norm_pre(dst_ap, pps, "ps"))

        # ---------------- Phase A: K^T (rope+norm) and V ----------------
        for i in range(_NT):
            isl = slice(i * _TW, (i + 1) * _TW)
            for h in range(_HPG):
                hsl = slice(h * 128, (h + 1) * 128)
                p_k = pps.tile([128, _TW], f32, tag="ps")
                for c in range(_NCB):
                    nc.tensor.matmul(p_k[:], t_wk[c][:, hsl], t_xt[c][:, isl],
                                     start=(c == 0), stop=(c == _NCB - 1))
                nc.scalar.copy(t_kn[h][:, isl], p_k[:])
        # V-projs emitted here: independent PE work that fills the gaps in
        # the serial rope+norm chains below
        for tb in range(_NKC):
            bsl = slice(tb * 128, (tb + 1) * 128)
            p_v = ppo.tile([128, _HD], f32, tag="po")
            for c in range(_NCB):
                nc.tensor.matmul(p_v[:], t_xt[c][:, bsl], t_wv[c][:],
                                 start=(c == 0), stop=(c == _NCB - 1))
            nc.scalar.copy(t_v[tb][:], p_v[:])
        # stage-batched across heads: three chains in flight, each using its
        # own bcast-psum pool (ppq/ppd are otherwise idle in phase A)
        for i in range(_NT):
            isl = slice(i * _TW, (i + 1) * _TW)
            for h in range(_HPG):
                rope_part(t_kn[h][:, isl], i * _TW)
            bcs = []
            for h in range(_HPG):
                pool, tag = _bc_routes[h]
                bcs.append(norm_pre(t_kn[h][:, isl], pool, tag))
            for h in range(_HPG):
                norm_post(t_kn[h][:, isl], bcs[h])

        # ---------------- Phase B: per T_q tile ----------------
        a_ctr = [0]

        def q_chain(qt, h):
            qsl = slice(qt * _TW, (qt + 1) * _TW)
            hsl = slice(h * 128, (h + 1) * 128)
            p_q = ppq.tile([128, _TW], f32, tag="pq", name="p_q")
            for c in range(_NCB):
                nc.tensor.matmul(p_q[:], t_wq[c][:, hsl], t_xt[c][:, qsl],
                                 start=(c == 0), stop=(c == _NCB - 1))
            t_g = pg.tile([128, _TW], f32r, tag="g", name="g", bufs=7)
            nc.vector.tensor_copy(t_g[:], p_q[:])
            rope_part(t_g[:], qt * _TW)
            pool, tag = _bc_routes[h] if h < 2 else (pps, "ps")
            norm_post(t_g[:], norm_pre(t_g[:], pool, tag, ms_on_act=False))
            return t_g

        def attention(qt, h, t_g):
            """Causal attention for one (T_q tile, head). The den/AV matmuls
            are emitted LOOKAHEAD chunks behind the S/exp pair: the PE stream
            is in-order, so den(kc) stalls on exp(kc) unless later S-matmuls
            are issued first."""
            hsl = slice(h * 128, (h + 1) * 128)
            nchunk = 4 * qt + 4
            LOOKAHEAD = 3
            p_den = ppd.tile([1, _TW], f32, tag="pd", name="p_den")
            p_o = ppo.tile([128, _TW], f32, tag="po", name="p_o")
            a_tiles = {}

            def emit_s(kc):
                roff = 0 if kc < 4 * qt else (kc - 4 * qt) * 128
                nsl = slice(roff, _TW)
                ksl = slice(kc * 128, (kc + 1) * 128)
                p_s = pps.tile([128, _TW], f32, tag="ps", name="p_s")
                nc.tensor.matmul(p_s[:, nsl], t_kn[h][:, ksl], t_g[:, nsl],
                                 start=True, stop=True)
                t_a = pc.tile([128, _TW], f32r, tag=f"wk{a_ctr[0] % _NCB}",
                              name=f"a{a_ctr[0] % _NCB}")
                a_ctr[0] += 1
                nc.scalar.activation(t_a[:, nsl], p_s[:, nsl], Act.Exp,
                                     scale=1.0 / float(np.sqrt(_D)))
                if kc >= 4 * qt:  # diagonal chunk: triangular mask
                    dsl = slice(roff, roff + 128)
                    nc.vector.tensor_mul(t_a[:, dsl], t_a[:, dsl], t_tri[:])
                a_tiles[kc] = t_a

            def emit_acc(kc):
                roff = 0 if kc < 4 * qt else (kc - 4 * qt) * 128
                nsl = slice(roff, _TW)
                t_a = a_tiles.pop(kc)
                nc.tensor.matmul(p_den[:, nsl], t_ones_col, t_a[:, nsl],
                                 start=(kc == 0), stop=(kc == nchunk - 1))
                nc.tensor.matmul(p_o[:, nsl], t_v[kc][:, hsl], t_a[:, nsl],
                                 start=(kc == 0), stop=(kc == nchunk - 1))

            for kc in range(nchunk + LOOKAHEAD):
                if kc < nchunk:
                    emit_s(kc)
                if kc >= LOOKAHEAD:
                    emit_acc(kc - LOOKAHEAD)
            # normalize: Z = O_unnorm * (1/den) broadcast
            t_den = psm.tile([1, _TW], f32r, tag="ms", name="t_den", bufs=3)
            nc.scalar.copy(t_den[:], p_den[:])
            p_db = pps.tile([128, _TW], f32, tag="ps", name="p_db")
            nc.tensor.matmul(p_db[:], t_ones_row, t_den[:], start=True, stop=True)
            t_rc2 = psm.tile([128, _TW], f32r, tag="sd", name="t_rc2", bufs=3)
            nc.vector.reciprocal(t_rc2[:], p_db[:])
            zi = h + _HPG * (qt % 2)
            t_z = pc.tile([128, _TW], f32r, tag=f"wv{zi}", name=f"z{zi}")
            nc.vector.tensor_mul(t_z[:], p_o[:], t_rc2[:])
            return t_z

        for qt in range(_NT):
            gs = [q_chain(qt, h) for h in range(_HPG)]
            z_chunks = [attention(qt, h, gs[h]) for h in range(_HPG)]
            # c_proj for this tile: partial out rows [qt*512, qt*512+512)
            for tb in range(4):
                bsl = slice(tb * 128, (tb + 1) * 128)
                t_ob = pob.tile([128, _C], f32, tag="ob")
                for nh in range(2):
                    osl = slice(nh * 384, (nh + 1) * 384)
                    p_c = ppq.tile([128, 384], f32, tag="pq")
                    for c in range(_HPG):
                        nc.tensor.matmul(p_c[:], z_chunks[c][:, bsl],
                                         t_wo[c][:, osl],
                                         start=(c == 0), stop=(c == _HPG - 1))
                    nc.vector.tensor_copy(t_ob[:, osl], p_c[:])
                nc.sync.dma_start(
                    out[qt * _TW + tb * 128: qt * _TW + (tb + 1) * 128, :],
                    t_ob[:])

    nc.compile()
    return nc


def _get_nc():
    if "nc" not in _cached:
        _cached["nc"] = _build_nc()
    return _cached["nc"]


def make_in_maps(x, cos, sin, Wq, Wk, Wv, Wo):
    cosT = np.ascontiguousarray(cos.reshape(_T, _D // 2).T)  # (64, T)
    sinT = np.ascontiguousarray(sin.reshape(_T, _D // 2).T)
    cc = np.concatenate([cosT, cosT], axis=0)                # (128, T)
    ss = np.concatenate([sinT, -sinT], axis=0)
    tri = (np.arange(128)[None, :] >= np.arange(128)[:, None]).astype(np.float32)
    ones128 = np.ones((128, 128), dtype=np.float32)
    permm = np.zeros((128, 128), dtype=np.float32)           # half-swap permutation
    for d in range(64):
        permm[64 + d, d] = 1.0
        permm[d, 64 + d] = 1.0
    in_maps = []
    for core in range(8):
        b, g = divmod(core, 2)
        gsl = slice(g * _HD, (g + 1) * _HD)
        in_maps.append({
            "xT": np.ascontiguousarray(x[b].T),
            "wq": np.ascontiguousarray(Wq[gsl, :].T),
            "wk": np.ascontiguousarray(Wk[gsl, :].T),
            "wv": np.ascontiguousarray(Wv[gsl, :].T),
            "wo": np.ascontiguousarray(Wo[:, gsl].T),
            "cc": cc, "ss": ss, "tri": tri, "ones": ones128, "perm": permm,
        })
    return in_maps


def kernel(x, cos, sin, Wq, Wk, Wv, Wo):
    from concourse.bass_utils import run_bass_kernel_spmd

    x = np.asarray(x, dtype=np.float32)
    cos = np.asarray(cos, dtype=np.float32)
    sin = np.asarray(sin, dtype=np.float32)
    Wq = np.asarray(Wq, dtype=np.float32)
    Wk = np.asarray(Wk, dtype=np.float32)
    Wv = np.asarray(Wv, dtype=np.float32)
    Wo = np.asarray(Wo, dtype=np.float32)

    nc = _get_nc()
    in_maps = make_in_maps(x, cos, sin, Wq, Wk, Wv, Wo)
    res = run_bass_kernel_spmd(nc, in_maps, core_ids=list(range(8)))
    outs = [r_["out"] for r_ in res.results]
    return np.stack([outs[2 * b] + outs[2 * b + 1] for b in range(_B)], axis=0)



# revision 9
# speedup vs baseline: 1.3172x; 1.3172x over previous
"""Trainium2 Bass kernel for CausalSelfAttention (B=4, T=2048, C=768, H=6, D=128)
with RoPE + QK-RMSNorm.

Sharding: 8 cores = batch(4) x head-group(2, 3 heads each). Host sums the two
head-group c_proj partials per batch.

v2 design (vs baseline):
  - bf16 data plane (inputs, K/Q/V/A tiles): halves input DMA, enables DVE
    2x elementwise modes, and removes the f32r 4-cycle penalty on narrow
    (128-col) diagonal matmuls.
  - K's RMS-norm is never applied to the K tile: the per-k scale
    1/(rms_k*sqrt(D)) is folded into the exp's per-partition scale AP.
  - softmax denominator comes free from the AV matmul: AV is computed
    transposed (out [q,128d]) with a ones column appended to V (129 cols),
    so column 128 accumulates sum_k(A). No separate den matmuls, and den
    lands per-partition (per-q) for a tensor_scalar normalize.
  - normalized z^T is transposed back via cheap PE transposes.
  - c_proj partials DMA directly from PSUM to DRAM (no staging copies).
"""

import numpy as np

_B, _T, _C, _H, _D = 4, 2048, 768, 6, 128
_HPG = 3            # heads per group
_HD = _HPG * _D     # 384, per-group head dims
_NT = 4             # T tiles of 512
_TW = 512           # tile width (T_q)
_NKC = _T // 128    # 16 k-chunks of 128
_NCB = _C // 128    # 6 c_in chunks
_EPS = 1e-15

_cached = {}


def _build_nc():
    from contextlib import ExitStack
    from concourse import bacc, tile, mybir

    f32 = mybir.dt.float32
    f32r = mybir.dt.float32r
    bf16 = mybir.dt.bfloat16
    Act = mybir.ActivationFunctionType

    nc = bacc.Bacc("TRN2", target_bir_lowering=False, debug=False)

    xT = nc.dram_tensor("xT", (_C, _T), bf16, kind="ExternalInput").ap()
    wq = nc.dram_tensor("wq", (_C, _HD), bf16, kind="ExternalInput").ap()
    wk = nc.dram_tensor("wk", (_C, _HD), bf16, kind="ExternalInput").ap()
    wv = nc.dram_tensor("wv", (_C, _HD), bf16, kind="ExternalInput").ap()
    wo = nc.dram_tensor("wo", (_HD, _C), bf16, kind="ExternalInput").ap()
    cc = nc.dram_tensor("cc", (128, _T), bf16, kind="ExternalInput").ap()
    ss = nc.dram_tensor("ss", (128, _T), bf16, kind="ExternalInput").ap()
    # cst16: [tri(128) | ident(128) | ones_col(1)] in bf16
    cst = nc.dram_tensor("cst", (128, 257), bf16, kind="ExternalInput").ap()
    perm = nc.dram_tensor("perm", (128, 128), bf16, kind="ExternalInput").ap()
    onesr = nc.dram_tensor("onesr", (1, 128), f32r, kind="ExternalInput").ap()
    out = nc.dram_tensor("out", (_T, _C), bf16, kind="ExternalOutput").ap()

    with tile.TileContext(nc) as tc, ExitStack() as ctx, \
            nc.allow_low_precision(reason="bf16 data plane; psum accumulate f32"):
        # --- pools ---
        pc = ctx.enter_context(tc.tile_pool(name="pc", bufs=1))
        pg = ctx.enter_context(tc.tile_pool(name="pg", bufs=3))        # scratch
        pa = ctx.enter_context(tc.tile_pool(name="pa", bufs=1))        # A tiles
        psm = ctx.enter_context(tc.tile_pool(name="psm", bufs=3))      # small rows
        # psum pools (8 banks total)
        pps = ctx.enter_context(tc.tile_pool(name="pps", bufs=2, space="PSUM"))
        ppo = ctx.enter_context(tc.tile_pool(name="ppo", bufs=2, space="PSUM"))
        ppq = ctx.enter_context(tc.tile_pool(name="ppq", bufs=2, space="PSUM"))
        ppd = ctx.enter_context(tc.tile_pool(name="ppd", bufs=1, space="PSUM"))
        ppm = ctx.enter_context(tc.tile_pool(name="ppm", bufs=1, space="PSUM"))

        # --- inputs resident in SBUF (load order: first-needed first) ---
        t_wk, t_xt, t_wv, t_wq = [], [], [], []
        for c in range(_NCB):
            t = pc.tile([128, _HD], bf16, tag=f"wk{c}", name=f"wk{c}")
            nc.sync.dma_start(t[:], wk[c * 128:(c + 1) * 128, :])
            t_wk.append(t)
        for c in range(_NCB):
            t = pc.tile([128, _T], bf16, tag=f"xt{c}", name=f"xt{c}")
            nc.sync.dma_start(t[:], xT[c * 128:(c + 1) * 128, :])
            t_xt.append(t)
        for c in range(_NCB):
            t = pc.tile([128, _HD], bf16, tag=f"wv{c}", name=f"wv{c}")
            nc.sync.dma_start(t[:], wv[c * 128:(c + 1) * 128, :])
            t_wv.append(t)
        t_cc = pc.tile([128, _T], bf16, tag="cc")
        t_ss = pc.tile([128, _T], bf16, tag="ss")
        nc.sync.dma_start(t_cc[:], cc[:])
        nc.sync.dma_start(t_ss[:], ss[:])
        t_cst = pc.tile([128, 257], bf16, tag="cst")
        t_perm = pc.tile([128, 128], bf16, tag="perm")
        t_onesr = pc.tile([1, 128], f32r, tag="onesr")
        nc.sync.dma_start(t_cst[:], cst[:])
        nc.sync.dma_start(t_perm[:], perm[:])
        nc.sync.dma_start(t_onesr[:], onesr[:])
        for c in range(_NCB):
            t = pc.tile([128, _HD], bf16, tag=f"wq{c}", name=f"wq{c}")
            nc.sync.dma_start(t[:], wq[c * 128:(c + 1) * 128, :])
            t_wq.append(t)
        t_wo = []
        for c in range(_HPG):
            t = pc.tile([128, _C], bf16, tag=f"wo{c}", name=f"wo{c}")
            nc.sync.dma_start(t[:], wo[c * 128:(c + 1) * 128, :])
            t_wo.append(t)

        t_tri = t_cst[:, 0:128]
        t_ident = t_cst[:, 128:256]
        t_ones_col = t_cst[:, 256:257]

        t_eps1 = pc.tile([1, 1], f32, tag="eps1")
        nc.gpsimd.memset(t_eps1[:], _EPS)
        t_eps128 = pc.tile([128, 1], f32, tag="eps128")
        nc.gpsimd.memset(t_eps128[:], 128.0 * _EPS)

        # persistent K^T (rope'd, UN-normalized) per head; V blocks with ones col
        t_kn = [pc.tile([128, _T], bf16, tag=f"kn{h}", name=f"kn{h}")
                for h in range(_HPG)]
        # t_vo[kc]: [V_h0 | ones | V_h1 | ones | V_h2 | ones] -> 129 per head
        t_vo = [pc.tile([128, 387], bf16, tag=f"vo{tb}", name=f"vo{tb}")
                for tb in range(_NKC)]
        # per-head exp scale columns: c_k = 1/sqrt(ms_k + 128 eps), per k
        t_ck = [pc.tile([128, _NKC], f32, tag=f"ck{h}", name=f"ck{h}")
                for h in range(_HPG)]
        # A tiles: 16 k-chunks x 3 heads, persistent within a qt iteration
        t_a = [[pa.tile([128, _TW], bf16, tag=f"a{h}_{kc}", name=f"a{h}_{kc}")
                for kc in range(_NKC)] for h in range(_HPG)]
        # z tiles per head: [d, q] layout for c_proj; zT is [q, d] pre-transpose
        t_zT = [pc.tile([128, _TW], bf16, tag=f"zT{h}", name=f"zT{h}")
                for h in range(_HPG)]
        t_z = [pc.tile([128, _TW], bf16, tag=f"z{h}", name=f"z{h}")
               for h in range(_HPG)]

        def rope(dst_ap, col0):
            """In-place RoPE on dst_ap (128, 512) bf16 sbuf tile slice."""
            csl = slice(col0, col0 + _TW)
            p_sw = ppq.tile([128, _TW], f32, tag="pq", name="p_sw")
            nc.tensor.matmul(p_sw[:], t_perm[:], dst_ap, start=True, stop=True)
            t_sw = pg.tile([128, _TW], bf16, tag="sw", name="t_sw")
            nc.vector.tensor_mul(dst_ap, dst_ap, t_cc[:, csl])
            nc.vector.tensor_mul(t_sw[:], p_sw[:], t_ss[:, csl])
            nc.vector.tensor_add(dst_ap, dst_ap, t_sw[:])

        # ---------------- Phase A: K^T (rope, stats) and V ----------------
        for i in range(_NT):
            isl = slice(i * _TW, (i + 1) * _TW)
            for h in range(_HPG):
                hsl = slice(h * 128, (h + 1) * 128)
                p_k = pps.tile([128, _TW], f32, tag="ps", name="p_k")
                for c in range(_NCB):
                    nc.tensor.matmul(p_k[:], t_wk[c][:, hsl], t_xt[c][:, isl],
                                     start=(c == 0), stop=(c == _NCB - 1))
                nc.scalar.copy(t_kn[h][:, isl], p_k[:])
        # V-projections: independent PE work overlapping the rope chains
        for tb in range(_NKC):
            bsl = slice(tb * 128, (tb + 1) * 128)
            p_v = ppo.tile([128, _HD], f32, tag="po", name="p_v")
            for c in range(_NCB):
                nc.tensor.matmul(p_v[:], t_xt[c][:, bsl], t_wv[c][:],
                                 start=(c == 0), stop=(c == _NCB - 1))
            # strided copy into [V_h | ones] layout + memset of ones columns
            vo_v = t_vo[tb][:].rearrange("p (h d) -> p h d", h=_HPG)[:, :, 0:128]
            pv_v = p_v[:].rearrange("p (h d) -> p h d", h=_HPG)
            nc.scalar.copy(vo_v, pv_v)
            nc.gpsimd.memset(t_vo[tb][:].rearrange("p (h d) -> p h d", h=_HPG)
                             [:, :, 128:129], 1.0)
        # rope + norm-stats on K, stage-batched across heads
        for i in range(_NT):
            isl = slice(i * _TW, (i + 1) * _TW)
            for h in range(_HPG):
                rope(t_kn[h][:, isl], i * _TW)
            for h in range(_HPG):
                t_sq = pg.tile([128, _TW], bf16, tag="sq", name="t_sq")
                nc.vector.tensor_mul(t_sq[:], t_kn[h][:, isl], t_kn[h][:, isl])
                p_msk = ppd.tile([128, 4], f32, tag="pd", name="p_msk")
                for j in range(4):
                    nc.tensor.matmul(p_msk[:, j:j + 1],
                                     t_sq[:, j * 128:(j + 1) * 128],
                                     t_ones_col,
                                     start=True, stop=True)
                t_sd = pg.tile([128, 4], f32, tag="sd4", name="t_sd4")
                nc.scalar.activation(t_sd[:], p_msk[:], Act.Sqrt,
                                     bias=t_eps128[:], scale=1.0)
                nc.vector.reciprocal(t_ck[h][:, i * 4:i * 4 + 4], t_sd[:])

        # ---------------- Phase B: per T_q tile ----------------
        def q_chain(qt, h):
            qsl = slice(qt * _TW, (qt + 1) * _TW)
            hsl = slice(h * 128, (h + 1) * 128)
            p_q = ppq.tile([128, _TW], f32, tag="pq", name="p_q")
            for c in range(_NCB):
                nc.tensor.matmul(p_q[:], t_wq[c][:, hsl], t_xt[c][:, qsl],
                                 start=(c == 0), stop=(c == _NCB - 1))
            t_g = pg.tile([128, _TW], bf16, tag=f"g{h}", name=f"g{h}", bufs=2)
            nc.scalar.copy(t_g[:], p_q[:])
            rope(t_g[:], qt * _TW)
            # RMS-norm: ms row -> sqrt -> recip -> broadcast -> scale
            t_sq = pg.tile([128, _TW], bf16, tag="sq", name="t_sq")
            nc.vector.tensor_mul(t_sq[:], t_g[:], t_g[:])
            p_ms = ppm.tile([1, _TW], f32, tag="pms", name="p_ms")
            nc.tensor.matmul(p_ms[:], t_ones_col, t_sq[:], start=True, stop=True)
            t_sd = psm.tile([1, _TW], f32, tag="sd", name="t_sd")
            nc.scalar.activation(t_sd[:], p_ms[:], Act.Sqrt,
                                 bias=t_eps1[:], scale=1.0 / 128.0)
            t_rq = psm.tile([1, _TW], f32r, tag="rq", name="t_rq")
            nc.vector.reciprocal(t_rq[:], t_sd[:])
            p_bc = ppd.tile([128, _TW], f32, tag="pd", name="p_bc")
            nc.tensor.matmul(p_bc[:], t_onesr[:], t_rq[:], start=True, stop=True)
            nc.vector.tensor_mul(t_g[:], t_g[:], p_bc[:])
            return t_g

        for qt in range(_NT):
            nchunk = 4 * qt + 4
            gs = [q_chain(qt, h) for h in range(_HPG)]
            # S + exp, interleaved across heads (PE stays dense; Act trails)
            for kc in range(nchunk):
                roff = 0 if kc < 4 * qt else (kc - 4 * qt) * 128
                nsl = slice(roff, _TW)
                ksl = slice(kc * 128, (kc + 1) * 128)
                for h in range(_HPG):
                    p_s = pps.tile([128, _TW], f32, tag="ps", name="p_s")
                    nc.tensor.matmul(p_s[:, nsl], t_kn[h][:, ksl], gs[h][:, nsl],
                                     start=True, stop=True)
                    nc.scalar.activation(t_a[h][kc][:, nsl], p_s[:, nsl],
                                         Act.Exp, scale=t_ck[h][:, kc:kc + 1])
                    if kc >= 4 * qt:  # diagonal chunk: triangular mask
                        dsl = slice(roff, roff + 128)
                        nc.vector.tensor_mul(t_a[h][kc][:, dsl],
                                             t_a[h][kc][:, dsl], t_tri)
            # AV transposed, with den in column 128; then normalize+transpose
            for h in range(_HPG):
                vsl = slice(h * 129, (h + 1) * 129)
                p_z2 = pps.tile([128, _TW], bf16, tag="ps", name="p_z2",
                                padded_shape=[128, 2 * _TW])
                for qb in range(4):
                    qbsl = slice(qb * 128, (qb + 1) * 128)
                    kmax = 4 * qt + qb
                    p_ot = ppo.tile([128, 129], f32, tag="po", name="p_ot")
                    for kc in range(kmax + 1):
                        nc.tensor.matmul(p_ot[:], t_a[h][kc][:, qbsl],
                                         t_vo[kc][:, vsl],
                                         start=(kc == 0), stop=(kc == kmax))
                    t_rd = psm.tile([128, 1], f32, tag="rd", name="t_rd")
                    nc.vector.reciprocal(t_rd[:], p_ot[:, 128:129])
                    nc.vector.tensor_scalar_mul(t_zT[h][:, qbsl],
                                                p_ot[:, 0:128], t_rd[:])
                    nc.tensor.transpose(p_z2[:, qbsl], t_zT[h][:, qbsl],
                                        t_ident)
                nc.vector.tensor_copy(t_z[h][:], p_z2[:])
            # c_proj: partial out rows, DMA directly from PSUM
            for tb in range(4):
                bsl = slice(tb * 128, (tb + 1) * 128)
                rsl = slice(qt * _TW + tb * 128, qt * _TW + (tb + 1) * 128)
                t_ob = pg.tile([128, _C], bf16, tag="ob", name="t_ob", bufs=2)
                for nh in range(2):
                    osl = slice(nh * 384, (nh + 1) * 384)
                    p_c = ppq.tile([128, 384], f32, tag="pq", name="p_c")
                    for c in range(_HPG):
                        nc.tensor.matmul(p_c[:], t_z[c][:, bsl],
                                         t_wo[c][:, osl],
                                         start=(c == 0), stop=(c == _HPG - 1))
                    if nh == 0:
                        nc.scalar.copy(t_ob[:, osl], p_c[:])
                    else:
                        nc.vector.tensor_copy(t_ob[:, osl], p_c[:])
                nc.sync.dma_start(out[rsl, :], t_ob[:])

    nc.compile()
    return nc


def _get_nc():
    if "nc" not in _cached:
        _cached["nc"] = _build_nc()
    return _cached["nc"]


def make_in_maps(x, cos, sin, Wq, Wk, Wv, Wo):
    import ml_dtypes
    bf = ml_dtypes.bfloat16

    cosT = np.ascontiguousarray(cos.reshape(_T, _D // 2).T)  # (64, T)
    sinT = np.ascontiguousarray(sin.reshape(_T, _D // 2).T)
    cc = np.concatenate([cosT, cosT], axis=0)                # (128, T)
    ss = np.concatenate([sinT, -sinT], axis=0)
    tri = (np.arange(128)[None, :] >= np.arange(128)[:, None]).astype(np.float32)
    ident = np.eye(128, dtype=np.float32)
    cst = np.concatenate([tri, ident, np.ones((128, 1), np.float32)], axis=1)
    permm = np.zeros((128, 128), dtype=np.float32)           # half-swap perm
    for d in range(64):
        permm[64 + d, d] = 1.0
        permm[d, 64 + d] = 1.0
    onesr = np.ones((1, 128), dtype=np.float32)
    in_maps = []
    for core in range(8):
        b, g = divmod(core, 2)
        gsl = slice(g * _HD, (g + 1) * _HD)
        in_maps.append({
            "xT": np.ascontiguousarray(x[b].T).astype(bf),
            "wq": np.ascontiguousarray(Wq[gsl, :].T).astype(bf),
            "wk": np.ascontiguousarray(Wk[gsl, :].T).astype(bf),
            "wv": np.ascontiguousarray(Wv[gsl, :].T).astype(bf),
            "wo": np.ascontiguousarray(Wo[:, gsl].T).astype(bf),
            "cc": cc.astype(bf), "ss": ss.astype(bf),
            "cst": cst.astype(bf), "perm": permm.astype(bf),
            "onesr": onesr,
        })
    return in_maps


def kernel(x, cos, sin, Wq, Wk, Wv, Wo):
    from concourse.bass_utils import run_bass_kernel_spmd

    x = np.asarray(x, dtype=np.float32)
    cos = np.asarray(cos, dtype=np.float32)
    sin = np.asarray(sin, dtype=np.float32)
    Wq = np.asarray(Wq, dtype=np.float32)
    Wk = np.asarray(Wk, dtype=np.float32)
    Wv = np.asarray(Wv, dtype=np.float32)
    Wo = np.asarray(Wo, dtype=np.float32)

    nc = _get_nc()
    in_maps = make_in_maps(x, cos, sin, Wq, Wk, Wv, Wo)
    res = run_bass_kernel_spmd(nc, in_maps, core_ids=list(range(8)))
    outs = [np.asarray(r_["out"], dtype=np.float32) for r_ in res.results]
    return np.stack([outs[2 * b] + outs[2 * b + 1] for b in range(_B)], axis=0)


# revision 10
# speedup vs baseline: 1.5100x; 1.1464x over previous
"""Trainium2 Bass kernel for CausalSelfAttention (B=4, T=2048, C=768, H=6, D=128)
with RoPE + QK-RMSNorm.

Sharding: 8 cores = batch(4) x head-group(2, 3 heads each). Host sums the two
head-group c_proj partials per batch.

v3 design:
  - bf16 data plane (inputs, K/Q/V/A tiles): halves input DMA, enables DVE
    2x elementwise modes, removes the f32r 4-cycle penalty on narrow matmuls.
  - K's RMS-norm is folded into the exp's per-partition scale AP (never
    applied to the K tile).
  - softmax denominator comes free from the AV matmul: AV is computed
    transposed (out [q,129]) with a ones column appended to V, so column 128
    accumulates sum_k(A); normalized z^T transposes back via PE transposes.
  - phase A computes ALL projections + RoPE + norms (Q tiles persistent), so
    phase B's Activation engine runs Exp only -- no act-table reloads.
  - phase B interleaves qt's AV/tails/c_proj with qt+1's S/exp stream.
"""

import numpy as np

_B, _T, _C, _H, _D = 4, 2048, 768, 6, 128
_HPG = 3            # heads per group
_HD = _HPG * _D     # 384, per-group head dims
_NT = 4             # T tiles of 512
_TW = 512           # tile width (T_q)
_NKC = _T // 128    # 16 k-chunks of 128
_NCB = _C // 128    # 6 c_in chunks
_EPS = 1e-15

_cached = {}


def _build_nc():
    from contextlib import ExitStack
    from concourse import bacc, tile, mybir

    f32 = mybir.dt.float32
    f32r = mybir.dt.float32r
    bf16 = mybir.dt.bfloat16
    Act = mybir.ActivationFunctionType

    nc = bacc.Bacc("TRN2", target_bir_lowering=False, debug=False)

    xT = nc.dram_tensor("xT", (_C, _T), bf16, kind="ExternalInput").ap()
    wq = nc.dram_tensor("wq", (_C, _HD), bf16, kind="ExternalInput").ap()
    wk = nc.dram_tensor("wk", (_C, _HD), bf16, kind="ExternalInput").ap()
    wv = nc.dram_tensor("wv", (_C, _HD), bf16, kind="ExternalInput").ap()
    wo = nc.dram_tensor("wo", (_HD, _C), bf16, kind="ExternalInput").ap()
    cc = nc.dram_tensor("cc", (128, _T), bf16, kind="ExternalInput").ap()
    ss = nc.dram_tensor("ss", (128, _T), bf16, kind="ExternalInput").ap()
    # cst: [tri(128) | ident(128) | ones_col(1)] in bf16
    cst = nc.dram_tensor("cst", (128, 257), bf16, kind="ExternalInput").ap()
    perm = nc.dram_tensor("perm", (128, 128), bf16, kind="ExternalInput").ap()
    onesr = nc.dram_tensor("onesr", (1, 128), f32r, kind="ExternalInput").ap()
    out = nc.dram_tensor("out", (_T, _C), bf16, kind="ExternalOutput").ap()

    with tile.TileContext(nc) as tc, ExitStack() as ctx, \
            nc.allow_low_precision(reason="bf16 data plane; psum accumulate f32"):
        # --- pools ---
        pc = ctx.enter_context(tc.tile_pool(name="pc", bufs=1))
        pg = ctx.enter_context(tc.tile_pool(name="pg", bufs=3))        # scratch
        pa = ctx.enter_context(tc.tile_pool(name="pa", bufs=1))        # A tiles
        psm = ctx.enter_context(tc.tile_pool(name="psm", bufs=3))      # small rows
        # psum pools (8 banks total)
        pps = ctx.enter_context(tc.tile_pool(name="pps", bufs=2, space="PSUM"))
        ppo = ctx.enter_context(tc.tile_pool(name="ppo", bufs=2, space="PSUM"))
        ppq = ctx.enter_context(tc.tile_pool(name="ppq", bufs=2, space="PSUM"))
        ppd = ctx.enter_context(tc.tile_pool(name="ppd", bufs=1, space="PSUM"))
        ppm = ctx.enter_context(tc.tile_pool(name="ppm", bufs=1, space="PSUM"))

        # --- inputs resident in SBUF (load order: first-needed first) ---
        t_wk, t_xt, t_wv, t_wq = [], [], [], []
        for c in range(_NCB):
            t = pc.tile([128, _HD], bf16, tag=f"wk{c}", name=f"wk{c}")
            nc.sync.dma_start(t[:], wk[c * 128:(c + 1) * 128, :])
            t_wk.append(t)
        for c in range(_NCB):
            t = pc.tile([128, _T], bf16, tag=f"xt{c}", name=f"xt{c}")
            nc.sync.dma_start(t[:], xT[c * 128:(c + 1) * 128, :])
            t_xt.append(t)
        for c in range(_NCB):
            t = pc.tile([128, _HD], bf16, tag=f"wv{c}", name=f"wv{c}")
            nc.sync.dma_start(t[:], wv[c * 128:(c + 1) * 128, :])
            t_wv.append(t)
        for c in range(_NCB):
            t = pc.tile([128, _HD], bf16, tag=f"wq{c}", name=f"wq{c}")
            nc.sync.dma_start(t[:], wq[c * 128:(c + 1) * 128, :])
            t_wq.append(t)
        t_cc = pc.tile([128, _T], bf16, tag="cc")
        t_ss = pc.tile([128, _T], bf16, tag="ss")
        nc.sync.dma_start(t_cc[:], cc[:])
        nc.sync.dma_start(t_ss[:], ss[:])
        t_cst = pc.tile([128, 257], bf16, tag="cst")
        t_perm = pc.tile([128, 128], bf16, tag="perm")
        t_onesr = pc.tile([1, 128], f32r, tag="onesr")
        nc.sync.dma_start(t_cst[:], cst[:])
        nc.sync.dma_start(t_perm[:], perm[:])
        nc.sync.dma_start(t_onesr[:], onesr[:])
        t_wo = []
        for c in range(_HPG):
            t = pc.tile([128, _C], bf16, tag=f"wo{c}", name=f"wo{c}")
            nc.sync.dma_start(t[:], wo[c * 128:(c + 1) * 128, :])
            t_wo.append(t)

        t_tri = t_cst[:, 0:128]
        t_ident = t_cst[:, 128:256]
        t_ones_col = t_cst[:, 256:257]

        t_eps1 = pc.tile([1, 1], f32, tag="eps1")
        nc.gpsimd.memset(t_eps1[:], _EPS)
        t_eps128 = pc.tile([128, 1], f32, tag="eps128")
        nc.gpsimd.memset(t_eps128[:], 128.0 * _EPS)

        # persistent K^T (rope'd, UN-normalized) per head; V blocks w/ ones col
        t_kn = [pc.tile([128, _T], bf16, tag=f"kn{h}", name=f"kn{h}")
                for h in range(_HPG)]
        t_vo = [pc.tile([128, 387], bf16, tag=f"vo{tb}", name=f"vo{tb}")
                for tb in range(_NKC)]
        # per-head exp scale columns: c_k = 1/sqrt(ms_k + 128 eps), per k
        t_ck = [pc.tile([128, _NKC], f32, tag=f"ck{h}", name=f"ck{h}")
                for h in range(_HPG)]
        # all Q tiles (rope'd + normalized), persistent through phase B
        t_q = [[pc.tile([128, _TW], bf16, tag=f"q{qt}_{h}", name=f"q{qt}_{h}")
                for h in range(_HPG)] for qt in range(_NT)]
        # A tiles: 16 k-chunks x 3 heads, reused across qt
        t_a = [[pa.tile([128, _TW], bf16, tag=f"a{h}_{kc}", name=f"a{h}_{kc}")
                for kc in range(_NKC)] for h in range(_HPG)]
        t_zT = [pc.tile([128, _TW], bf16, tag=f"zT{h}", name=f"zT{h}")
                for h in range(_HPG)]
        t_z = [pc.tile([128, _TW], bf16, tag=f"z{h}", name=f"z{h}")
               for h in range(_HPG)]

        def rope(dst_ap, col0):
            """In-place RoPE on dst_ap (128, 512) bf16 sbuf tile slice.
            cc-mul on Pool (sbuf-only), psum-mul + add on DVE."""
            csl = slice(col0, col0 + _TW)
            p_sw = ppq.tile([128, _TW], f32, tag="pq", name="p_sw")
            nc.tensor.matmul(p_sw[:], t_perm[:], dst_ap, start=True, stop=True)
            t_sw = pg.tile([128, _TW], bf16, tag="sw", name="t_sw")
            nc.gpsimd.tensor_mul(dst_ap, dst_ap, t_cc[:, csl])
            nc.vector.tensor_mul(t_sw[:], p_sw[:], t_ss[:, csl])
            nc.vector.tensor_add(dst_ap, dst_ap, t_sw[:])

        # ============ Phase A: projections, RoPE, norms ============
        for i in range(_NT):
            isl = slice(i * _TW, (i + 1) * _TW)
            for h in range(_HPG):
                hsl = slice(h * 128, (h + 1) * 128)
                p_k = pps.tile([128, _TW], f32, tag="ps", name="p_k")
                for c in range(_NCB):
                    nc.tensor.matmul(p_k[:], t_wk[c][:, hsl], t_xt[c][:, isl],
                                     start=(c == 0), stop=(c == _NCB - 1))
                nc.scalar.copy(t_kn[h][:, isl], p_k[:])
        for tb in range(_NKC):
            bsl = slice(tb * 128, (tb + 1) * 128)
            p_v = ppo.tile([128, _HD], f32, tag="po", name="p_v")
            for c in range(_NCB):
                nc.tensor.matmul(p_v[:], t_xt[c][:, bsl], t_wv[c][:],
                                 start=(c == 0), stop=(c == _NCB - 1))
            vo_v = t_vo[tb][:].rearrange("p (h d) -> p h d", h=_HPG)[:, :, 0:128]
            pv_v = p_v[:].rearrange("p (h d) -> p h d", h=_HPG)
            nc.scalar.copy(vo_v, pv_v)
            nc.gpsimd.memset(t_vo[tb][:].rearrange("p (h d) -> p h d", h=_HPG)
                             [:, :, 128:129], 1.0)
        # K rope + norm stats (c_k columns for the exp scale)
        for i in range(_NT):
            isl = slice(i * _TW, (i + 1) * _TW)
            for h in range(_HPG):
                rope(t_kn[h][:, isl], i * _TW)
            for h in range(_HPG):
                t_sq = pg.tile([128, _TW], bf16, tag="sq", name="t_sq")
                nc.vector.tensor_mul(t_sq[:], t_kn[h][:, isl], t_kn[h][:, isl])
                p_msk = ppd.tile([128, 4], f32, tag="pd", name="p_msk")
                for j in range(4):
                    nc.tensor.matmul(p_msk[:, j:j + 1],
                                     t_sq[:, j * 128:(j + 1) * 128],
                                     t_ones_col, start=True, stop=True)
                t_sd4 = pg.tile([128, 4], f32, tag="sd4", name="t_sd4")
                nc.scalar.activation(t_sd4[:], p_msk[:], Act.Sqrt,
                                     bias=t_eps128[:], scale=1.0)
                nc.vector.reciprocal(t_ck[h][:, i * 4:i * 4 + 4], t_sd4[:])
        # Q projections + rope + RMS-norm, all qt upfront
        for qt in range(_NT):
            qsl = slice(qt * _TW, (qt + 1) * _TW)
            for h in range(_HPG):
                hsl = slice(h * 128, (h + 1) * 128)
                p_q = ppq.tile([128, _TW], f32, tag="pq", name="p_q")
                for c in range(_NCB):
                    nc.tensor.matmul(p_q[:], t_wq[c][:, hsl], t_xt[c][:, qsl],
                                     start=(c == 0), stop=(c == _NCB - 1))
                nc.scalar.copy(t_q[qt][h][:], p_q[:])
            for h in range(_HPG):
                rope(t_q[qt][h][:], qt * _TW)
            for h in range(_HPG):
                t_sq = pg.tile([128, _TW], bf16, tag="sq", name="t_sq")
                nc.vector.tensor_mul(t_sq[:], t_q[qt][h][:], t_q[qt][h][:])
                p_ms = ppm.tile([1, _TW], f32, tag="pms", name="p_ms")
                nc.tensor.matmul(p_ms[:], t_ones_col, t_sq[:],
                                 start=True, stop=True)
                t_sd = psm.tile([1, _TW], f32, tag="sd", name="t_sd")
                nc.scalar.activation(t_sd[:], p_ms[:], Act.Sqrt,
                                     bias=t_eps1[:], scale=1.0 / 128.0)
                t_rq = psm.tile([1, _TW], f32r, tag="rq", name="t_rq")
                nc.vector.reciprocal(t_rq[:], t_sd[:])
                p_bc = ppd.tile([128, _TW], f32, tag="pd", name="p_bc")
                nc.tensor.matmul(p_bc[:], t_onesr[:], t_rq[:],
                                 start=True, stop=True)
                nc.vector.tensor_mul(t_q[qt][h][:], t_q[qt][h][:], p_bc[:])

        # ============ Phase B: attention + c_proj, qt-pipelined ============
        def emit_s_exp(qt):
            nchunk = 4 * qt + 4
            for kc in range(nchunk):
                roff = 0 if kc < 4 * qt else (kc - 4 * qt) * 128
                nsl = slice(roff, _TW)
                ksl = slice(kc * 128, (kc + 1) * 128)
                for h in range(_HPG):
                    p_s = pps.tile([128, _TW], f32, tag="ps", name="p_s")
                    nc.tensor.matmul(p_s[:, nsl], t_kn[h][:, ksl],
                                     t_q[qt][h][:, nsl], start=True, stop=True)
                    nc.scalar.activation(t_a[h][kc][:, nsl], p_s[:, nsl],
                                         Act.Exp, scale=t_ck[h][:, kc:kc + 1])
                    if kc >= 4 * qt:  # diagonal chunk: triangular mask
                        dsl = slice(roff, roff + 128)
                        nc.gpsimd.tensor_mul(t_a[h][kc][:, dsl],
                                             t_a[h][kc][:, dsl], t_tri)

        def emit_av(qt):
            for h in range(_HPG):
                vsl = slice(h * 129, (h + 1) * 129)
                for qb in range(4):
                    qbsl = slice(qb * 128, (qb + 1) * 128)
                    kmax = 4 * qt + qb
                    p_ot = ppo.tile([128, 129], f32, tag="po", name="p_ot")
                    for kc in range(kmax + 1):
                        nc.tensor.matmul(p_ot[:], t_a[h][kc][:, qbsl],
                                         t_vo[kc][:, vsl],
                                         start=(kc == 0), stop=(kc == kmax))
                    t_rd = psm.tile([128, 1], f32, tag="rd", name="t_rd")
                    nc.vector.reciprocal(t_rd[:], p_ot[:, 128:129])
                    nc.vector.tensor_scalar_mul(t_zT[h][:, qbsl],
                                                p_ot[:, 0:128], t_rd[:])

        def emit_tail_cproj(qt):
            for h in range(_HPG):
                p_z2 = pps.tile([128, _TW], bf16, tag="ps", name="p_z2",
                                padded_shape=[128, 2 * _TW])
                for qb in range(4):
                    qbsl = slice(qb * 128, (qb + 1) * 128)
                    nc.tensor.transpose(p_z2[:, qbsl], t_zT[h][:, qbsl],
                                        t_ident)
                nc.vector.tensor_copy(t_z[h][:], p_z2[:])
            for tb in range(4):
                bsl = slice(tb * 128, (tb + 1) * 128)
                rsl = slice(qt * _TW + tb * 128, qt * _TW + (tb + 1) * 128)
                t_ob = pg.tile([128, _C], bf16, tag="ob", name="t_ob", bufs=2)
                for nh in range(2):
                    osl = slice(nh * 384, (nh + 1) * 384)
                    p_c = ppq.tile([128, 384], f32, tag="pq", name="p_c")
                    for c in range(_HPG):
                        nc.tensor.matmul(p_c[:], t_z[c][:, bsl],
                                         t_wo[c][:, osl],
                                         start=(c == 0), stop=(c == _HPG - 1))
                    nc.vector.tensor_copy(t_ob[:, osl], p_c[:])
                nc.sync.dma_start(out[rsl, :], t_ob[:])

        emit_s_exp(0)
        for qt in range(_NT):
            emit_av(qt)
            if qt + 1 < _NT:
                emit_s_exp(qt + 1)
            emit_tail_cproj(qt)

    nc.compile()
    return nc


def _get_nc():
    if "nc" not in _cached:
        _cached["nc"] = _build_nc()
    return _cached["nc"]


def make_in_maps(x, cos, sin, Wq, Wk, Wv, Wo):
    import ml_dtypes
    bf = ml_dtypes.bfloat16

    cosT = np.ascontiguousarray(cos.reshape(_T, _D // 2).T)  # (64, T)
    sinT = np.ascontiguousarray(sin.reshape(_T, _D // 2).T)
    cc = np.concatenate([cosT, cosT], axis=0)                # (128, T)
    ss = np.concatenate([sinT, -sinT], axis=0)
    tri = (np.arange(128)[None, :] >= np.arange(128)[:, None]).astype(np.float32)
    ident = np.eye(128, dtype=np.float32)
    cst = np.concatenate([tri, ident, np.ones((128, 1), np.float32)], axis=1)
    permm = np.zeros((128, 128), dtype=np.float32)           # half-swap perm
    for d in range(64):
        permm[64 + d, d] = 1.0
        permm[d, 64 + d] = 1.0
    onesr = np.ones((1, 128), dtype=np.float32)
    in_maps = []
    for core in range(8):
        b, g = divmod(core, 2)
        gsl = slice(g * _HD, (g + 1) * _HD)
        in_maps.append({
            "xT": np.ascontiguousarray(x[b].T).astype(bf),
            "wq": np.ascontiguousarray(Wq[gsl, :].T).astype(bf),
            "wk": np.ascontiguousarray(Wk[gsl, :].T).astype(bf),
            "wv": np.ascontiguousarray(Wv[gsl, :].T).astype(bf),
            "wo": np.ascontiguousarray(Wo[:, gsl].T).astype(bf),
            "cc": cc.astype(bf), "ss": ss.astype(bf),
            "cst": cst.astype(bf), "perm": permm.astype(bf),
            "onesr": onesr,
        })
    return in_maps


def kernel(x, cos, sin, Wq, Wk, Wv, Wo):
    from concourse.bass_utils import run_bass_kernel_spmd

    x = np.asarray(x, dtype=np.float32)
    cos = np.asarray(cos, dtype=np.float32)
    sin = np.asarray(sin, dtype=np.float32)
    Wq = np.asarray(Wq, dtype=np.float32)
    Wk = np.asarray(Wk, dtype=np.float32)
    Wv = np.asarray(Wv, dtype=np.float32)
    Wo = np.asarray(Wo, dtype=np.float32)

    nc = _get_nc()
    in_maps = make_in_maps(x, cos, sin, Wq, Wk, Wv, Wo)
    res = run_bass_kernel_spmd(nc, in_maps, core_ids=list(range(8)))
    outs = [np.asarray(r_["out"], dtype=np.float32) for r_ in res.results]
    return np.stack([outs[2 * b] + outs[2 * b + 1] for b in range(_B)], axis=0)


# revision 22
# speedup vs baseline: 1.5267x; 1.0110x over previous
"""Trainium2 Bass kernel for CausalSelfAttention (B=4, T=2048, C=768, H=6, D=128)
with RoPE + QK-RMSNorm.

Sharding: 8 cores = batch(4) x head-group(2, 3 heads each). Host sums the two
head-group c_proj partials per batch.

v3 design:
  - bf16 data plane (inputs, K/Q/V/A tiles): halves input DMA, enables DVE
    2x elementwise modes, removes the f32r 4-cycle penalty on narrow matmuls.
  - K's RMS-norm is folded into the exp's per-partition scale AP (never
    applied to the K tile).
  - softmax denominator comes free from the AV matmul: AV is computed
    transposed (out [q,129]) with a ones column appended to V, so column 128
    accumulates sum_k(A); normalized z^T transposes back via PE transposes.
  - phase A computes ALL projections + RoPE + norms (Q tiles persistent), so
    phase B's Activation engine runs Exp only -- no act-table reloads.
  - phase B interleaves qt's AV/tails/c_proj with qt+1's S/exp stream.
"""

import numpy as np

_B, _T, _C, _H, _D = 4, 2048, 768, 6, 128
_HPG = 3            # heads per group
_HD = _HPG * _D     # 384, per-group head dims
_NT = 4             # T tiles of 512
_TW = 512           # tile width (T_q)
_NKC = _T // 128    # 16 k-chunks of 128
_NCB = _C // 128    # 6 c_in chunks
_EPS = 1e-15

_cached = {}


def _build_nc():
    from contextlib import ExitStack
    from concourse import bacc, tile, mybir

    f32 = mybir.dt.float32
    f32r = mybir.dt.float32r
    bf16 = mybir.dt.bfloat16
    Act = mybir.ActivationFunctionType

    nc = bacc.Bacc("TRN2", target_bir_lowering=False, debug=False)

    xT = nc.dram_tensor("xT", (_C, _T), bf16, kind="ExternalInput").ap()
    wq = nc.dram_tensor("wq", (_C, _HD), bf16, kind="ExternalInput").ap()
    wk = nc.dram_tensor("wk", (_C, _HD), bf16, kind="ExternalInput").ap()
    wv = nc.dram_tensor("wv", (_C, _HD), bf16, kind="ExternalInput").ap()
    wo = nc.dram_tensor("wo", (_HD, _C), bf16, kind="ExternalInput").ap()
    cc = nc.dram_tensor("cc", (128, _T), bf16, kind="ExternalInput").ap()
    ss = nc.dram_tensor("ss", (128, _T), bf16, kind="ExternalInput").ap()
    # cst: [tri(128) | ident(128) | ones_col(1)] in bf16
    cst = nc.dram_tensor("cst", (128, 257), bf16, kind="ExternalInput").ap()
    perm = nc.dram_tensor("perm", (128, 128), bf16, kind="ExternalInput").ap()
    onesr = nc.dram_tensor("onesr", (1, 128), bf16, kind="ExternalInput").ap()
    out = nc.dram_tensor("out", (_T, _C), bf16, kind="ExternalOutput").ap()

    with tile.TileContext(nc) as tc, ExitStack() as ctx, \
            nc.allow_low_precision(reason="bf16 data plane; psum accumulate f32"):
        # --- pools ---
        pc = ctx.enter_context(tc.tile_pool(name="pc", bufs=1))
        pg = ctx.enter_context(tc.tile_pool(name="pg", bufs=3))        # scratch
        pa = ctx.enter_context(tc.tile_pool(name="pa", bufs=1))        # A tiles
        psm = ctx.enter_context(tc.tile_pool(name="psm", bufs=3))      # small rows
        # psum pools (8 banks total)
        pps = ctx.enter_context(tc.tile_pool(name="pps", bufs=2, space="PSUM"))
        ppo = ctx.enter_context(tc.tile_pool(name="ppo", bufs=2, space="PSUM"))
        ppq = ctx.enter_context(tc.tile_pool(name="ppq", bufs=2, space="PSUM"))
        ppd = ctx.enter_context(tc.tile_pool(name="ppd", bufs=1, space="PSUM"))
        ppm = ctx.enter_context(tc.tile_pool(name="ppm", bufs=1, space="PSUM"))

        # --- inputs resident in SBUF (load order: first-needed first) ---
        t_wk, t_xt, t_wv, t_wq = [], [], [], []
        for c in range(_NCB):
            t = pc.tile([128, _HD], bf16, tag=f"wk{c}", name=f"wk{c}")
            nc.sync.dma_start(t[:], wk[c * 128:(c + 1) * 128, :])
            t_wk.append(t)
        for c in range(_NCB):
            t = pc.tile([128, _T], bf16, tag=f"xt{c}", name=f"xt{c}")
            nc.sync.dma_start(t[:], xT[c * 128:(c + 1) * 128, :])
            t_xt.append(t)
        for c in range(_NCB):
            t = pc.tile([128, _HD], bf16, tag=f"wv{c}", name=f"wv{c}")
            nc.sync.dma_start(t[:], wv[c * 128:(c + 1) * 128, :])
            t_wv.append(t)
        for c in range(_NCB):
            t = pc.tile([128, _HD], bf16, tag=f"wq{c}", name=f"wq{c}")
            nc.sync.dma_start(t[:], wq[c * 128:(c + 1) * 128, :])
            t_wq.append(t)
        t_cc = pc.tile([128, _T], bf16, tag="cc")
        t_ss = pc.tile([128, _T], bf16, tag="ss")
        nc.sync.dma_start(t_cc[:], cc[:])
        nc.sync.dma_start(t_ss[:], ss[:])
        t_cst = pc.tile([128, 257], bf16, tag="cst")
        t_perm = pc.tile([128, 128], bf16, tag="perm")
        t_onesr = pc.tile([1, 128], bf16, tag="onesr")
        nc.sync.dma_start(t_cst[:], cst[:])
        nc.sync.dma_start(t_perm[:], perm[:])
        nc.sync.dma_start(t_onesr[:], onesr[:])
        t_wo = []
        for c in range(_HPG):
            t = pc.tile([128, _C], bf16, tag=f"wo{c}", name=f"wo{c}")
            nc.sync.dma_start(t[:], wo[c * 128:(c + 1) * 128, :])
            t_wo.append(t)

        t_tri = t_cst[:, 0:128]
        t_ident = t_cst[:, 128:256]
        t_ones_col = t_cst[:, 256:257]

        t_epsA = pc.tile([128, 1], f32, tag="epsA")   # K: 128*eps
        nc.gpsimd.memset(t_epsA[:], 128.0 * _EPS)
        t_epsB = pc.tile([128, 1], f32, tag="epsB")   # Q: eps
        nc.gpsimd.memset(t_epsB[:], _EPS)

        # persistent K^T (rope'd, UN-normalized) per head; V blocks w/ ones col
        t_kn = [pc.tile([128, _T], bf16, tag=f"kn{h}", name=f"kn{h}")
                for h in range(_HPG)]
        t_vo = [pc.tile([128, 387], bf16, tag=f"vo{tb}", name=f"vo{tb}")
                for tb in range(_NKC)]
        # exp scale columns: c_k = 1/sqrt(ms_k + 128 eps); col h*16+kc
        t_ck = pc.tile([128, _HPG * _NKC], f32, tag="ck", name="t_ck")
        # Q-norm reciprocal columns: col (qt*3+h)*4+qb
        t_cqb = pc.tile([128, 48], bf16, tag="cqb", name="t_cqb")
        # all Q tiles (rope'd + normalized), persistent through phase B
        t_q = [[pc.tile([128, _TW], bf16, tag=f"q{qt}_{h}", name=f"q{qt}_{h}")
                for h in range(_HPG)] for qt in range(_NT)]
        # A tiles: 16 k-chunks x 3 heads, reused across qt
        t_a = [[pa.tile([128, _TW], bf16, tag=f"a{h}_{kc}", name=f"a{h}_{kc}")
                for kc in range(_NKC)] for h in range(_HPG)]
        t_zT = [pc.tile([128, _TW], bf16, tag=f"zT{h}", name=f"zT{h}")
                for h in range(_HPG)]
        t_z = [pc.tile([128, _TW], bf16, tag=f"z{h}", name=f"z{h}")
               for h in range(_HPG)]

        def rope(dst_ap, col0):
            """In-place RoPE on dst_ap (128, 512) bf16 sbuf tile slice.
            cc-mul on Pool (sbuf-only), psum-mul + add on DVE."""
            csl = slice(col0, col0 + _TW)
            p_sw = ppq.tile([128, _TW], f32, tag="pq", name="p_sw")
            nc.tensor.matmul(p_sw[:], t_perm[:], dst_ap, start=True, stop=True)
            t_sw = pg.tile([128, _TW], bf16, tag="sw", name="t_sw")
            nc.gpsimd.tensor_mul(dst_ap, dst_ap, t_cc[:, csl])
            nc.vector.tensor_mul(t_sw[:], p_sw[:], t_ss[:, csl])
            nc.vector.tensor_add(dst_ap, dst_ap, t_sw[:])

        # ============ Phase A: projections, RoPE, norms ============
        for i in range(_NT):
            isl = slice(i * _TW, (i + 1) * _TW)
            for h in range(_HPG):
                hsl = slice(h * 128, (h + 1) * 128)
                p_k = pps.tile([128, _TW], f32, tag="ps", name="p_k")
                for c in range(_NCB):
                    nc.tensor.matmul(p_k[:], t_wk[c][:, hsl], t_xt[c][:, isl],
                                     start=(c == 0), stop=(c == _NCB - 1))
                nc.scalar.copy(t_kn[h][:, isl], p_k[:])
        for tb in range(_NKC):
            bsl = slice(tb * 128, (tb + 1) * 128)
            p_v = ppo.tile([128, _HD], f32, tag="po", name="p_v")
            for c in range(_NCB):
                nc.tensor.matmul(p_v[:], t_xt[c][:, bsl], t_wv[c][:],
                                 start=(c == 0), stop=(c == _NCB - 1))
            vo_v = t_vo[tb][:].rearrange("p (h d) -> p h d", h=_HPG)[:, :, 0:128]
            pv_v = p_v[:].rearrange("p (h d) -> p h d", h=_HPG)
            nc.scalar.copy(vo_v, pv_v)
            nc.gpsimd.memset(t_vo[tb][:].rearrange("p (h d) -> p h d", h=_HPG)
                             [:, :, 128:129], 1.0)
        # K rope + norm stats, batched into one [128,48] psum -> 1 Sqrt/recip
        p_msk = ppd.tile([128, _HPG * _NKC], f32, tag="pd", name="p_msk")
        for i in range(_NT):
            isl = slice(i * _TW, (i + 1) * _TW)
            for h in range(_HPG):
                rope(t_kn[h][:, isl], i * _TW)
            for h in range(_HPG):
                t_sq = pg.tile([128, _TW], bf16, tag="sq", name="t_sq")
                nc.vector.tensor_mul(t_sq[:], t_kn[h][:, isl], t_kn[h][:, isl])
                for j in range(4):
                    col = h * _NKC + i * 4 + j
                    nc.tensor.matmul(p_msk[:, col:col + 1],
                                     t_sq[:, j * 128:(j + 1) * 128],
                                     t_ones_col, start=True, stop=True)
        t_sd48 = pg.tile([128, _HPG * _NKC], f32, tag="sd48", name="t_sd48")
        nc.scalar.activation(t_sd48[:], p_msk[:], Act.Sqrt,
                             bias=t_epsA[:], scale=1.0)
        nc.vector.reciprocal(t_ck[:], t_sd48[:])
        # Q projections + rope + RMS-norm, qt descending. ms in column
        # layout ([128,48] psum, one Sqrt + one recip), then per-(qt,h)
        # mini-transposes rebuild the broadcast row.
        for qt in reversed(range(_NT)):
            qsl = slice(qt * _TW, (qt + 1) * _TW)
            for h in range(_HPG):
                hsl = slice(h * 128, (h + 1) * 128)
                p_q = ppq.tile([128, _TW], f32, tag="pq", name="p_q")
                for c in range(_NCB):
                    nc.tensor.matmul(p_q[:], t_wq[c][:, hsl], t_xt[c][:, qsl],
                                     start=(c == 0), stop=(c == _NCB - 1))
                nc.scalar.copy(t_q[qt][h][:], p_q[:])
        p_msq = ppd.tile([128, 48], f32, tag="pd", name="p_msq")
        for qt in reversed(range(_NT)):
            for h in range(_HPG):
                rope(t_q[qt][h][:], qt * _TW)
        for qt in reversed(range(_NT)):
            for h in range(_HPG):
                t_sq = pg.tile([128, _TW], bf16, tag="sq", name="t_sq")
                nc.vector.tensor_mul(t_sq[:], t_q[qt][h][:], t_q[qt][h][:])
                for qb in range(4):
                    col = (qt * _HPG + h) * 4 + qb
                    nc.tensor.matmul(p_msq[:, col:col + 1],
                                     t_sq[:, qb * 128:(qb + 1) * 128],
                                     t_ones_col, start=True, stop=True)
        t_sd48b = pg.tile([128, 48], f32, tag="sd48", name="t_sd48b")
        nc.scalar.activation(t_sd48b[:], p_msq[:], Act.Sqrt,
                             bias=t_epsB[:], scale=1.0 / 128.0)
        nc.vector.reciprocal(t_cqb[:], t_sd48b[:])
        for qt in reversed(range(_NT)):
            for h in range(_HPG):
                base = (qt * _HPG + h) * 4
                p_rq = ppm.tile([1, _TW], bf16, tag="pms", name="p_rq",
                                padded_shape=[1, 2 * _TW])
                for qb in range(4):
                    nc.tensor.transpose(p_rq[0:1, qb * 128:(qb + 1) * 128],
                                        t_cqb[:, base + qb:base + qb + 1],
                                        t_ident)
                t_rqr = psm.tile([1, _TW], bf16, tag="rqr", name="t_rqr")
                nc.vector.tensor_copy(t_rqr[:], p_rq[:])
                p_bc = ppd.tile([128, _TW], f32, tag="pd", name="p_bc")
                nc.tensor.matmul(p_bc[:], t_onesr[:], t_rqr[:],
                                 start=True, stop=True)
                nc.vector.tensor_mul(t_q[qt][h][:], t_q[qt][h][:], p_bc[:])

        # ============ Phase B: attention + c_proj, qt-pipelined ============
        def emit_s_exp(qt):
            nchunk = 4 * qt + 4
            for kc in range(nchunk):
                roff = 0 if kc < 4 * qt else (kc - 4 * qt) * 128
                nsl = slice(roff, _TW)
                ksl = slice(kc * 128, (kc + 1) * 128)
                for h in range(_HPG):
                    p_s = pps.tile([128, _TW], f32, tag="ps", name="p_s")
                    nc.tensor.matmul(p_s[:, nsl], t_kn[h][:, ksl],
                                     t_q[qt][h][:, nsl], start=True, stop=True)
                    ckc = h * _NKC + kc
                    nc.scalar.activation(t_a[h][kc][:, nsl], p_s[:, nsl],
                                         Act.Exp,
                                         scale=t_ck[:, ckc:ckc + 1])
                    if kc >= 4 * qt:  # diagonal chunk: triangular mask
                        dsl = slice(roff, roff + 128)
                        nc.gpsimd.tensor_mul(t_a[h][kc][:, dsl],
                                             t_a[h][kc][:, dsl], t_tri)

        def emit_av(qt):
            for h in range(_HPG):
                vsl = slice(h * 129, (h + 1) * 129)
                for qb in range(4):
                    qbsl = slice(qb * 128, (qb + 1) * 128)
                    kmax = 4 * qt + qb
                    p_ot = ppo.tile([128, 129], f32, tag="po", name="p_ot")
                    for kc in range(kmax + 1):
                        nc.tensor.matmul(p_ot[:], t_a[h][kc][:, qbsl],
                                         t_vo[kc][:, vsl],
                                         start=(kc == 0), stop=(kc == kmax))
                    t_rd = psm.tile([128, 1], f32, tag="rd", name="t_rd")
                    nc.vector.reciprocal(t_rd[:], p_ot[:, 128:129])
                    nc.vector.tensor_scalar_mul(t_zT[h][:, qbsl],
                                                p_ot[:, 0:128], t_rd[:])

        def emit_tail_cproj(qt):
            for h in range(_HPG):
                p_z2 = pps.tile([128, _TW], bf16, tag="ps", name="p_z2",
                                padded_shape=[128, 2 * _TW])
                for qb in range(4):
                    qbsl = slice(qb * 128, (qb + 1) * 128)
                    nc.tensor.transpose(p_z2[:, qbsl], t_zT[h][:, qbsl],
                                        t_ident)
                nc.vector.tensor_copy(t_z[h][:], p_z2[:])
            for tb in range(4):
                bsl = slice(tb * 128, (tb + 1) * 128)
                rsl = slice(qt * _TW + tb * 128, qt * _TW + (tb + 1) * 128)
                t_ob = pg.tile([128, _C], bf16, tag="ob", name="t_ob", bufs=2)
                for nh in range(2):
                    osl = slice(nh * 384, (nh + 1) * 384)
                    p_c = ppq.tile([128, 384], f32, tag="pq", name="p_c")
                    for c in range(_HPG):
                        nc.tensor.matmul(p_c[:], t_z[c][:, bsl],
                                         t_wo[c][:, osl],
                                         start=(c == 0), stop=(c == _HPG - 1))
                    nc.vector.tensor_copy(t_ob[:, osl], p_c[:])
                nc.sync.dma_start(out[rsl, :], t_ob[:])

        qt_order = list(reversed(range(_NT)))
        emit_s_exp(qt_order[0])
        for n, qt in enumerate(qt_order):
            emit_av(qt)
            if n + 1 < _NT:
                emit_s_exp(qt_order[n + 1])
            emit_tail_cproj(qt)

    nc.compile()
    return nc


def _get_nc():
    if "nc" not in _cached:
        _cached["nc"] = _build_nc()
    return _cached["nc"]


def make_in_maps(x, cos, sin, Wq, Wk, Wv, Wo):
    import ml_dtypes
    bf = ml_dtypes.bfloat16

    cosT = np.ascontiguousarray(cos.reshape(_T, _D // 2).T)  # (64, T)
    sinT = np.ascontiguousarray(sin.reshape(_T, _D // 2).T)
    cc = np.concatenate([cosT, cosT], axis=0)                # (128, T)
    ss = np.concatenate([sinT, -sinT], axis=0)
    tri = (np.arange(128)[None, :] >= np.arange(128)[:, None]).astype(np.float32)
    ident = np.eye(128, dtype=np.float32)
    cst = np.concatenate([tri, ident, np.ones((128, 1), np.float32)], axis=1)
    permm = np.zeros((128, 128), dtype=np.float32)           # half-swap perm
    for d in range(64):
        permm[64 + d, d] = 1.0
        permm[d, 64 + d] = 1.0
    onesr = np.ones((1, 128), dtype=np.float32)
    in_maps = []
    for core in range(8):
        b, g = divmod(core, 2)
        gsl = slice(g * _HD, (g + 1) * _HD)
        in_maps.append({
            "xT": np.ascontiguousarray(x[b].T).astype(bf),
            "wq": np.ascontiguousarray(Wq[gsl, :].T).astype(bf),
            "wk": np.ascontiguousarray(Wk[gsl, :].T).astype(bf),
            "wv": np.ascontiguousarray(Wv[gsl, :].T).astype(bf),
            "wo": np.ascontiguousarray(Wo[:, gsl].T).astype(bf),
            "cc": cc.astype(bf), "ss": ss.astype(bf),
            "cst": cst.astype(bf), "perm": permm.astype(bf),
            "onesr": onesr.astype(bf),
        })
    return in_maps


def kernel(x, cos, sin, Wq, Wk, Wv, Wo):
    from concourse.bass_utils import run_bass_kernel_spmd

    x = np.asarray(x, dtype=np.float32)
    cos = np.asarray(cos, dtype=np.float32)
    sin = np.asarray(sin, dtype=np.float32)
    Wq = np.asarray(Wq, dtype=np.float32)
    Wk = np.asarray(Wk, dtype=np.float32)
    Wv = np.asarray(Wv, dtype=np.float32)
    Wo = np.asarray(Wo, dtype=np.float32)

    nc = _get_nc()
    in_maps = make_in_maps(x, cos, sin, Wq, Wk, Wv, Wo)
    res = run_bass_kernel_spmd(nc, in_maps, core_ids=list(range(8)))
    outs = [np.asarray(r_["out"], dtype=np.float32) for r_ in res.results]
    return np.stack([outs[2 * b] + outs[2 * b + 1] for b in range(_B)], axis=0)
